# revision 14
# baseline (speedup 1.0000x reference)
"""Chamfer loss (chunked) Trainium2 kernel — nn_ChamferLoss_8194797601432.

Reference math: for each 2048-point chunk c of pc1, compute the vector
min over the chunk of ||pc2[m] - pc1_chunk[p]||^2 for all m in pc2 (and
symmetrically for chunks of pc2 vs pc1), concatenate, return
mean(dist1) + mean(dist2)  (scalar fp32).

Device strategy (8 NeuronCores, SPMD, per-core data):
  Core c handles chunk c for both halves (2 tasks per core):
    task := (ref = full opposite cloud [16384 pts], chunk = 2048 pts)
    PE computes NEGATED shifted distances G'[m,p] = 2*ref[m]·chunk[p]
      - ||chunk[p]||² = -(d(m,p) - ||ref[m]||²) via K=12 bf16 rows
      (2-term bf16 split per coordinate + 3-term split of ||chunk||²:
      products exact in fp32, final rel err ~4e-5 vs the 2e-2 gate;
      fp32r's ~1e-3 noise biased the min low by ~9e-2 — the reason the
      original version failed).
    Work unit = one m-tile: 4 matmuls (512 chunk pts each) fill one
      [128, 2048] PSUM tile (4 banks); ONE VectorE pool_max drains it
      at ~2 elem/lane/cycle (the PSUM port's 8-byte read width)
      straight into minbuf[:, mt]. The two tasks' unit streams are
      interleaved so the PE fills task A's PSUM tile while VectorE
      pools task B's (2 tiles = all 8 banks); row-groups alternate
      (0,32) per unit so consecutive units' self-loading matmuls land
      on different PE quadrants and overlap. Measured 1078 ns per
      m-tile ≈ 99% of the 2048/2/0.96GHz pool-stream roofline; the
      276 us total is PSUM-drain-bound (ScalarE copies, scans, SBUF
      pools, and GPSIMD were all measured slower per element).
    Host: d_min = ||ref||² - maxout; concatenate, mean in float64.
"""

import numpy as np
import ml_dtypes

BF = ml_dtypes.bfloat16

NPTS = 16384
NCHUNK = 2048
NCORES = 8
NM = NPTS // 128  # 128 m-tiles per task
NTASKS = 2
K = 12  # bf16 feature rows
NSUB = 1  # sub-units (pools) per m-tile
NROT = 2  # row-group rotation
INTERLEAVE = True  # interleave the two tasks' units

_CACHE = {}


def _build(reps=1):
    import concourse.bacc as bacc
    import concourse.mybir as mybir
    import concourse.tile as tile
    from contextlib import ExitStack

    FP32 = mybir.dt.float32
    BF16 = mybir.dt.bfloat16
    MAXP = mybir.PoolFunctionType.max

    nc = bacc.Bacc("TRN2", target_bir_lowering=False)

    refs = [
        nc.dram_tensor(f"ref{t}", [K, NPTS], BF16, kind="ExternalInput")
        for t in range(NTASKS)
    ]
    chunks = [
        nc.dram_tensor(f"chunk{t}", [K, NCHUNK], BF16, kind="ExternalInput")
        for t in range(NTASKS)
    ]
    minout = nc.dram_tensor(
        "minout", [NTASKS, 128, NSUB * NM], FP32, kind="ExternalOutput"
    )

    with tile.TileContext(nc) as tc:
        with ExitStack() as ctx:
            const_pool = ctx.enter_context(tc.tile_pool(name="const", bufs=1))
            psum_pool = ctx.enter_context(
                tc.tile_pool(name="psum", bufs=8 * 512 * NSUB // NCHUNK, space="PSUM")
            )
            out_pool = ctx.enter_context(tc.tile_pool(name="out", bufs=1))

            Rs, Cs, minbufs = [], [], []
            for t in range(NTASKS):
                R = const_pool.tile([128, NPTS], BF16, tag=f"R{t}", name=f"R{t}")
                C = const_pool.tile([128, NCHUNK], BF16, tag=f"C{t}", name=f"C{t}")
                for g in range(NROT):
                    rg = 32 * g
                    nc.sync.dma_start(R[rg : rg + K, :], refs[t][:])
                    nc.sync.dma_start(C[rg : rg + K, :], chunks[t][:])
                Rs.append(R)
                Cs.append(C)
                minbufs.append(
                    out_pool.tile([128, NSUB * NM], FP32, tag=f"mb{t}", name=f"mb{t}")
                )

            loop_cm = tc.For_i(0, reps, 1) if reps > 1 else None
            if loop_cm is not None:
                loop_cm.__enter__()

            FD = NCHUNK // NSUB  # psum tile free dim
            NPS = FD // 512  # matmuls per sub-unit

            units = []
            for t in range(NTASKS):
                for mt in range(NM):
                    for su in range(NSUB):
                        units.append((t, mt, su))
            if INTERLEAVE:
                half = len(units) // 2
                units = [
                    units[h * half + i]
                    for i in range(half)
                    for h in range(2)
                ]

            for u, (t, mt, su) in enumerate(units):
                R, C, minbuf = Rs[t], Cs[t], minbufs[t]
                s = mt * NSUB + su
                rg = 32 * (u % NROT)
                lhsT = R[rg : rg + K, mt * 128 : (mt + 1) * 128]
                ps2 = psum_pool.tile([128, FD], FP32, tag="ps2", name=f"ps2_{t}_{s}")
                for p in range(NPS):
                    nc.tensor.matmul(
                        ps2[:, p * 512 : (p + 1) * 512],
                        lhsT=lhsT,
                        rhs=C[rg : rg + K, (su * NPS + p) * 512 : (su * NPS + p + 1) * 512],
                        start=True,
                        stop=True,
                        tile_position=(rg, 0),
                    )
                nc.vector.pool(minbuf[:, s : s + 1], ps2[:], func=MAXP)

            if loop_cm is not None:
                loop_cm.__exit__(None, None, None)

            for t in range(NTASKS):
                nc.sync.dma_start(minout[t], minbufs[t][:])

    nc.compile()
    return nc


def get_nc(reps=1):
    if reps not in _CACHE:
        _CACHE[reps] = _build(reps)
    return _CACHE[reps]


def _split3(x):
    """fp32 array -> three bf16 arrays with b0+b1+b2 ~ x (residual ~2^-27|x|)."""
    x = x.astype(np.float32)
    b0 = x.astype(BF)
    r1 = x - b0.astype(np.float32)
    b1 = r1.astype(BF)
    r2 = r1 - b1.astype(np.float32)
    b2 = r2.astype(BF)
    return b0, b1, b2


def _ref_feat(p):
    """p [N,3] fp32 -> [K, N] bf16 ref-side rows."""
    r0, r1, r2 = _split3(p.T)  # each [3, N]
    one = np.ones((p.shape[0],), BF)
    rows = []
    for d in range(3):
        rows += [r0[d], r0[d], r1[d]]
    rows += [one, one, one]
    return np.stack(rows)


def _chunk_feat(p):
    """p [n,3] fp32 -> [K, n] bf16 chunk-side rows (negated: G' = 2rc - v)."""
    s0, s1, s2 = _split3(2.0 * p.T)  # each [3, n]
    v = (p.astype(np.float64) ** 2).sum(-1).astype(np.float32)
    v0, v1, v2 = _split3(-v)
    rows = []
    for d in range(3):
        rows += [s0[d], s1[d], s0[d]]
    rows += [v0, v1, v2]
    return np.stack(rows)


def _prep_in_maps(pc1, pc2):
    refA = _ref_feat(pc2)  # dist1: ref = pc2, chunks of pc1
    refB = _ref_feat(pc1)  # dist2: ref = pc1, chunks of pc2
    in_maps = []
    for c in range(NCORES):
        in_maps.append(
            {
                "ref0": refA,
                "chunk0": _chunk_feat(pc1[c * NCHUNK : (c + 1) * NCHUNK]),
                "ref1": refB,
                "chunk1": _chunk_feat(pc2[c * NCHUNK : (c + 1) * NCHUNK]),
            }
        )
    return in_maps


def run_on_device(in_maps, reps=1):
    from concourse.bass_utils import run_bass_kernel_spmd

    nc = get_nc(reps)
    res = run_bass_kernel_spmd(nc, in_maps, core_ids=list(range(NCORES)))
    return res.results


def _postprocess(results, pc1, pc2):
    n2_1 = (pc1.astype(np.float64) ** 2).sum(-1)
    n2_2 = (pc2.astype(np.float64) ** 2).sum(-1)
    d1 = np.empty((NCORES, NPTS), np.float64)
    d2 = np.empty((NCORES, NPTS), np.float64)
    for c in range(NCORES):
        mo = results[c]["minout"].astype(np.float64)  # [2, 128, NSUB*NM]
        mx0 = mo[0].reshape(128, NM, NSUB).max(-1)  # [128, NM]
        mx1 = mo[1].reshape(128, NM, NSUB).max(-1)
        d1[c] = n2_2 - mx0.T.reshape(-1)
        d2[c] = n2_1 - mx1.T.reshape(-1)
    return np.array(d1.mean() + d2.mean(), dtype=np.float32)


def kernel(output_pc, gt_pc):
    pc1 = np.asarray(output_pc, dtype=np.float32).reshape(NPTS, 3)
    pc2 = np.asarray(gt_pc, dtype=np.float32).reshape(NPTS, 3)
    in_maps = _prep_in_maps(pc1, pc2)
    results = run_on_device(in_maps)
    return _postprocess(results, pc1, pc2)


# revision 15
# speedup vs baseline: 1.1027x; 1.1027x over previous
"""Chamfer loss (chunked) Trainium2 kernel — nn_ChamferLoss_8194797601432.

Reference math: for each 2048-point chunk c of pc1, compute the vector
min over the chunk of ||pc2[m] - pc1_chunk[p]||^2 for all m in pc2 (and
symmetrically for chunks of pc2 vs pc1), concatenate, return
mean(dist1) + mean(dist2)  (scalar fp32).

Device strategy (8 NeuronCores, SPMD, per-core data):
  Core c handles chunk c for both halves (2 tasks per core):
    task := (ref = full opposite cloud [16384 pts], chunk = 2048 pts)
    PE computes NEGATED shifted distances G'[m,p] = 2*ref[m]·chunk[p]
      - ||chunk[p]||² = -(d(m,p) - ||ref[m]||²) via K=12 bf16 rows
      (2-term bf16 split per coordinate + 3-term split of ||chunk||²:
      products exact in fp32, final rel err ~4e-5 vs the 2e-2 gate;
      fp32r's ~1e-3 noise biased the min low by ~9e-2 — the reason the
      original version failed).
    Work unit = one m-tile: 4 matmuls (512 chunk pts each) fill one
      [128, 2048] PSUM tile (4 banks); ONE VectorE pool_max drains it
      at ~2 elem/lane/cycle (the PSUM port's 8-byte read width)
      straight into minbuf[:, mt]. The two tasks' unit streams are
      interleaved so the PE fills task A's PSUM tile while VectorE
      pools task B's (2 tiles = all 8 banks); row-groups alternate
      (0,32) per unit so consecutive units' self-loading matmuls land
      on different PE quadrants and overlap. Measured 1078 ns per
      m-tile ≈ 99% of the 2048/2/0.96GHz pool-stream roofline; the
      276 us total is PSUM-drain-bound (ScalarE copies, scans, SBUF
      pools, and GPSIMD were all measured slower per element).
    Host: d_min = ||ref||² - maxout; concatenate, mean in float64.
"""

import numpy as np
import ml_dtypes

BF = ml_dtypes.bfloat16

NPTS = 16384
NCHUNK = 2048
NCORES = 8
NM = NPTS // 128  # 128 m-tiles per task
NTASKS = 2
K = 12  # bf16 feature rows
NSUB = 1  # sub-units (pools) per m-tile
NROT = 2  # row-group rotation
INTERLEAVE = True  # interleave the two tasks' units

_CACHE = {}


def _build(reps=1):
    import concourse.bacc as bacc
    import concourse.mybir as mybir
    import concourse.tile as tile
    from contextlib import ExitStack

    FP32 = mybir.dt.float32
    BF16 = mybir.dt.bfloat16
    MAXP = mybir.PoolFunctionType.max

    nc = bacc.Bacc("TRN2", target_bir_lowering=False)

    refs = [
        nc.dram_tensor(f"ref{t}", [K, NPTS], BF16, kind="ExternalInput")
        for t in range(NTASKS)
    ]
    chunks = [
        nc.dram_tensor(f"chunk{t}", [K, NCHUNK], BF16, kind="ExternalInput")
        for t in range(NTASKS)
    ]
    minout = nc.dram_tensor(
        "minout", [NTASKS, 128, NSUB * NM], FP32, kind="ExternalOutput"
    )

    with tile.TileContext(nc) as tc:
        with ExitStack() as ctx:
            const_pool = ctx.enter_context(tc.tile_pool(name="const", bufs=1))
            psum_pool = ctx.enter_context(
                tc.tile_pool(name="psum", bufs=8 * 512 * NSUB // NCHUNK, space="PSUM")
            )
            out_pool = ctx.enter_context(tc.tile_pool(name="out", bufs=1))

            Rs, Cs, minbufs = [], [], []
            for t in range(NTASKS):
                R = const_pool.tile([128, NPTS], BF16, tag=f"R{t}", name=f"R{t}")
                C = const_pool.tile([128, NCHUNK], BF16, tag=f"C{t}", name=f"C{t}")
                for g in range(NROT):
                    rg = 32 * g
                    nc.sync.dma_start(R[rg : rg + K, :], refs[t][:])
                    nc.sync.dma_start(C[rg : rg + K, :], chunks[t][:])
                Rs.append(R)
                Cs.append(C)
                minbufs.append(
                    out_pool.tile([128, NSUB * NM], FP32, tag=f"mb{t}", name=f"mb{t}")
                )

            loop_cm = tc.For_i(0, reps, 1) if reps > 1 else None
            if loop_cm is not None:
                loop_cm.__enter__()

            FD = NCHUNK // NSUB  # psum tile free dim
            NPS = FD // 512  # matmuls per sub-unit

            units = []
            for t in range(NTASKS):
                for mt in range(NM):
                    for su in range(NSUB):
                        units.append((t, mt, su))
            if INTERLEAVE:
                half = len(units) // 2
                units = [
                    units[h * half + i]
                    for i in range(half)
                    for h in range(2)
                ]

            for u, (t, mt, su) in enumerate(units):
                R, C, minbuf = Rs[t], Cs[t], minbufs[t]
                s = mt * NSUB + su
                ps2 = psum_pool.tile([128, FD], FP32, tag="ps2", name=f"ps2_{t}_{s}")
                for p in range(NPS):
                    # alternate row group per matmul: adjacent matmuls land on
                    # different PE quadrants and execute concurrently
                    rg = 32 * ((p + u) % NROT)
                    nc.tensor.matmul(
                        ps2[:, p * 512 : (p + 1) * 512],
                        lhsT=R[rg : rg + K, mt * 128 : (mt + 1) * 128],
                        rhs=C[rg : rg + K, (su * NPS + p) * 512 : (su * NPS + p + 1) * 512],
                        start=True,
                        stop=True,
                        tile_position=(rg, 0),
                    )
                nc.vector.pool(minbuf[:, s : s + 1], ps2[:], func=MAXP)

            if loop_cm is not None:
                loop_cm.__exit__(None, None, None)

            for t in range(NTASKS):
                nc.sync.dma_start(minout[t], minbufs[t][:])

    nc.compile()
    return nc


def get_nc(reps=1):
    if reps not in _CACHE:
        _CACHE[reps] = _build(reps)
    return _CACHE[reps]


def _split3(x):
    """fp32 array -> three bf16 arrays with b0+b1+b2 ~ x (residual ~2^-27|x|)."""
    x = x.astype(np.float32)
    b0 = x.astype(BF)
    r1 = x - b0.astype(np.float32)
    b1 = r1.astype(BF)
    r2 = r1 - b1.astype(np.float32)
    b2 = r2.astype(BF)
    return b0, b1, b2


def _ref_feat(p):
    """p [N,3] fp32 -> [K, N] bf16 ref-side rows."""
    r0, r1, r2 = _split3(p.T)  # each [3, N]
    one = np.ones((p.shape[0],), BF)
    rows = []
    for d in range(3):
        rows += [r0[d], r0[d], r1[d]]
    rows += [one, one, one]
    return np.stack(rows)


def _chunk_feat(p):
    """p [n,3] fp32 -> [K, n] bf16 chunk-side rows (negated: G' = 2rc - v)."""
    s0, s1, s2 = _split3(2.0 * p.T)  # each [3, n]
    v = (p.astype(np.float64) ** 2).sum(-1).astype(np.float32)
    v0, v1, v2 = _split3(-v)
    rows = []
    for d in range(3):
        rows += [s0[d], s1[d], s0[d]]
    rows += [v0, v1, v2]
    return np.stack(rows)


def _prep_in_maps(pc1, pc2):
    refA = _ref_feat(pc2)  # dist1: ref = pc2, chunks of pc1
    refB = _ref_feat(pc1)  # dist2: ref = pc1, chunks of pc2
    in_maps = []
    for c in range(NCORES):
        in_maps.append(
            {
                "ref0": refA,
                "chunk0": _chunk_feat(pc1[c * NCHUNK : (c + 1) * NCHUNK]),
                "ref1": refB,
                "chunk1": _chunk_feat(pc2[c * NCHUNK : (c + 1) * NCHUNK]),
            }
        )
    return in_maps


def run_on_device(in_maps, reps=1):
    from concourse.bass_utils import run_bass_kernel_spmd

    nc = get_nc(reps)
    res = run_bass_kernel_spmd(nc, in_maps, core_ids=list(range(NCORES)))
    return res.results


def _postprocess(results, pc1, pc2):
    n2_1 = (pc1.astype(np.float64) ** 2).sum(-1)
    n2_2 = (pc2.astype(np.float64) ** 2).sum(-1)
    d1 = np.empty((NCORES, NPTS), np.float64)
    d2 = np.empty((NCORES, NPTS), np.float64)
    for c in range(NCORES):
        mo = results[c]["minout"].astype(np.float64)  # [2, 128, NSUB*NM]
        mx0 = mo[0].reshape(128, NM, NSUB).max(-1)  # [128, NM]
        mx1 = mo[1].reshape(128, NM, NSUB).max(-1)
        d1[c] = n2_2 - mx0.T.reshape(-1)
        d2[c] = n2_1 - mx1.T.reshape(-1)
    return np.array(d1.mean() + d2.mean(), dtype=np.float32)


def kernel(output_pc, gt_pc):
    pc1 = np.asarray(output_pc, dtype=np.float32).reshape(NPTS, 3)
    pc2 = np.asarray(gt_pc, dtype=np.float32).reshape(NPTS, 3)
    in_maps = _prep_in_maps(pc1, pc2)
    results = run_on_device(in_maps)
    return _postprocess(results, pc1, pc2)


# revision 16
# speedup vs baseline: 1.1430x; 1.0365x over previous
"""Chamfer loss (chunked) Trainium2 kernel — nn_ChamferLoss_8194797601432.

Reference math: for each 2048-point chunk c of pc1, compute the vector
min over the chunk of ||pc2[m] - pc1_chunk[p]||^2 for all m in pc2 (and
symmetrically for chunks of pc2 vs pc1), concatenate, return
mean(dist1) + mean(dist2)  (scalar fp32).

Device strategy (8 NeuronCores, SPMD, per-core data):
  Core c handles chunk c for both halves (2 tasks per core):
    task := (ref = full opposite cloud [16384 pts], chunk = 2048 pts)
    PE computes NEGATED shifted distances G'[m,p] = 2*ref[m]·chunk[p]
      - ||chunk[p]||² = -(d(m,p) - ||ref[m]||²) via K=12 bf16 rows
      (2-term bf16 split per coordinate + 3-term split of ||chunk||²:
      products exact in fp32, final rel err ~4e-5 vs the 2e-2 gate;
      fp32r's ~1e-3 noise biased the min low by ~9e-2 — the reason the
      original version failed).
    Work unit = one m-tile: 4 matmuls (512 chunk pts each) fill one
      [128, 2048] PSUM tile (4 banks); ONE VectorE pool_max drains it
      at ~2 elem/lane/cycle (the PSUM port's 8-byte read width)
      straight into minbuf[:, mt]. The two tasks' unit streams are
      interleaved so the PE fills task A's PSUM tile while VectorE
      pools task B's (2 tiles = all 8 banks); row-groups alternate
      (0,32) per MATMUL so adjacent matmuls land on different PE
      quadrants and run concurrently — this hides the PE entirely
      even in the device's throttled sustained state. Measured
      2227 ns/m-tile sustained (4% above the throttled pool floor)
      and 1078 ns/m-tile in burst state ≈ 99% of the 2048/2/0.96GHz
      pool-stream roofline; PSUM-drain-bound (ScalarE copies, scans,
      SBUF pools, and GPSIMD all measured slower per element).
    Host: d_min = ||ref||² - maxout; concatenate, mean in float64.
"""

import numpy as np
import ml_dtypes

BF = ml_dtypes.bfloat16

NPTS = 16384
NCHUNK = 2048
NCORES = 8
NM = NPTS // 128  # 128 m-tiles per task
NTASKS = 2
K = 12  # bf16 feature rows
NSUB = 1  # sub-units (pools) per m-tile
NROT = 2  # row-group rotation
INTERLEAVE = True  # interleave the two tasks' units

_CACHE = {}


def _build(reps=1):
    import concourse.bacc as bacc
    import concourse.mybir as mybir
    import concourse.tile as tile
    from contextlib import ExitStack

    FP32 = mybir.dt.float32
    BF16 = mybir.dt.bfloat16
    MAXP = mybir.PoolFunctionType.max

    nc = bacc.Bacc("TRN2", target_bir_lowering=False)

    refs = [
        nc.dram_tensor(f"ref{t}", [K, NPTS], BF16, kind="ExternalInput")
        for t in range(NTASKS)
    ]
    chunks = [
        nc.dram_tensor(f"chunk{t}", [K, NCHUNK], BF16, kind="ExternalInput")
        for t in range(NTASKS)
    ]
    minout = nc.dram_tensor(
        "minout", [NTASKS, 128, NSUB * NM], FP32, kind="ExternalOutput"
    )

    with tile.TileContext(nc) as tc:
        with ExitStack() as ctx:
            const_pool = ctx.enter_context(tc.tile_pool(name="const", bufs=1))
            psum_pool = ctx.enter_context(
                tc.tile_pool(name="psum", bufs=8 * 512 * NSUB // NCHUNK, space="PSUM")
            )
            out_pool = ctx.enter_context(tc.tile_pool(name="out", bufs=1))

            Rs, Cs, minbufs = [], [], []
            for t in range(NTASKS):
                R = const_pool.tile([128, NPTS], BF16, tag=f"R{t}", name=f"R{t}")
                C = const_pool.tile([128, NCHUNK], BF16, tag=f"C{t}", name=f"C{t}")
                for g in range(NROT):
                    rg = 32 * g
                    nc.sync.dma_start(R[rg : rg + K, :], refs[t][:])
                    nc.sync.dma_start(C[rg : rg + K, :], chunks[t][:])
                Rs.append(R)
                Cs.append(C)
                minbufs.append(
                    out_pool.tile([128, NSUB * NM], FP32, tag=f"mb{t}", name=f"mb{t}")
                )

            loop_cm = tc.For_i(0, reps, 1) if reps > 1 else None
            if loop_cm is not None:
                loop_cm.__enter__()

            FD = NCHUNK // NSUB  # psum tile free dim
            NPS = FD // 512  # matmuls per sub-unit

            units = []
            for t in range(NTASKS):
                for mt in range(NM):
                    for su in range(NSUB):
                        units.append((t, mt, su))
            if INTERLEAVE:
                half = len(units) // 2
                units = [
                    units[h * half + i]
                    for i in range(half)
                    for h in range(2)
                ]

            for u, (t, mt, su) in enumerate(units):
                R, C, minbuf = Rs[t], Cs[t], minbufs[t]
                s = mt * NSUB + su
                ps2 = psum_pool.tile([128, FD], FP32, tag="ps2", name=f"ps2_{t}_{s}")
                for p in range(NPS):
                    # alternate row group per matmul: adjacent matmuls land on
                    # different PE quadrants and execute concurrently
                    rg = 32 * ((p + u) % NROT)
                    nc.tensor.matmul(
                        ps2[:, p * 512 : (p + 1) * 512],
                        lhsT=R[rg : rg + K, mt * 128 : (mt + 1) * 128],
                        rhs=C[rg : rg + K, (su * NPS + p) * 512 : (su * NPS + p + 1) * 512],
                        start=True,
                        stop=True,
                        tile_position=(rg, 0),
                    )
                nc.vector.pool(minbuf[:, s : s + 1], ps2[:], func=MAXP)

            if loop_cm is not None:
                loop_cm.__exit__(None, None, None)

            for t in range(NTASKS):
                nc.sync.dma_start(minout[t], minbufs[t][:])

    nc.compile()
    return nc


def get_nc(reps=1):
    if reps not in _CACHE:
        _CACHE[reps] = _build(reps)
    return _CACHE[reps]


def _split3(x):
    """fp32 array -> three bf16 arrays with b0+b1+b2 ~ x (residual ~2^-27|x|)."""
    x = x.astype(np.float32)
    b0 = x.astype(BF)
    r1 = x - b0.astype(np.float32)
    b1 = r1.astype(BF)
    r2 = r1 - b1.astype(np.float32)
    b2 = r2.astype(BF)
    return b0, b1, b2


def _ref_feat(p):
    """p [N,3] fp32 -> [K, N] bf16 ref-side rows."""
    r0, r1, r2 = _split3(p.T)  # each [3, N]
    one = np.ones((p.shape[0],), BF)
    rows = []
    for d in range(3):
        rows += [r0[d], r0[d], r1[d]]
    rows += [one, one, one]
    return np.stack(rows)


def _chunk_feat(p):
    """p [n,3] fp32 -> [K, n] bf16 chunk-side rows (negated: G' = 2rc - v)."""
    s0, s1, s2 = _split3(2.0 * p.T)  # each [3, n]
    v = (p.astype(np.float64) ** 2).sum(-1).astype(np.float32)
    v0, v1, v2 = _split3(-v)
    rows = []
    for d in range(3):
        rows += [s0[d], s1[d], s0[d]]
    rows += [v0, v1, v2]
    return np.stack(rows)


def _prep_in_maps(pc1, pc2):
    refA = _ref_feat(pc2)  # dist1: ref = pc2, chunks of pc1
    refB = _ref_feat(pc1)  # dist2: ref = pc1, chunks of pc2
    in_maps = []
    for c in range(NCORES):
        in_maps.append(
            {
                "ref0": refA,
                "chunk0": _chunk_feat(pc1[c * NCHUNK : (c + 1) * NCHUNK]),
                "ref1": refB,
                "chunk1": _chunk_feat(pc2[c * NCHUNK : (c + 1) * NCHUNK]),
            }
        )
    return in_maps


def run_on_device(in_maps, reps=1):
    from concourse.bass_utils import run_bass_kernel_spmd

    nc = get_nc(reps)
    res = run_bass_kernel_spmd(nc, in_maps, core_ids=list(range(NCORES)))
    return res.results


def _postprocess(results, pc1, pc2):
    n2_1 = (pc1.astype(np.float64) ** 2).sum(-1)
    n2_2 = (pc2.astype(np.float64) ** 2).sum(-1)
    d1 = np.empty((NCORES, NPTS), np.float64)
    d2 = np.empty((NCORES, NPTS), np.float64)
    for c in range(NCORES):
        mo = results[c]["minout"].astype(np.float64)  # [2, 128, NSUB*NM]
        mx0 = mo[0].reshape(128, NM, NSUB).max(-1)  # [128, NM]
        mx1 = mo[1].reshape(128, NM, NSUB).max(-1)
        d1[c] = n2_2 - mx0.T.reshape(-1)
        d2[c] = n2_1 - mx1.T.reshape(-1)
    return np.array(d1.mean() + d2.mean(), dtype=np.float32)


def kernel(output_pc, gt_pc):
    pc1 = np.asarray(output_pc, dtype=np.float32).reshape(NPTS, 3)
    pc2 = np.asarray(gt_pc, dtype=np.float32).reshape(NPTS, 3)
    in_maps = _prep_in_maps(pc1, pc2)
    results = run_on_device(in_maps)
    return _postprocess(results, pc1, pc2)
